# revision 2
# baseline (speedup 1.0000x reference)
"""Multi-head self-attention (B=1, S=4096, D=1024, H=16, causal) on 8 TRN2 cores.

Sharding: tensor-parallel over heads. Each core computes 2 heads end-to-end:
QKV projections (column-split), causal attention, and a partial output
projection (W_O row-split). Host sums the 8 partial outputs.

Per-core pipeline (fused, single NEFF, fp32r matmuls):
  for i in 0..7 (512-token tile):
    - load x s-block tiles, PE-transpose to xT
    - QT/KT/VT[:, tile] = W*T.T @ xT  (heads stacked on partitions)
    - PE-transpose VT tile -> V_ext [k,65] (ones column -> denominator)
    - causal attention for q-tile i over k-chunks 0..4i+3:
        S_T = KT_chunk.T @ QT_tile (2 heads row-packed), exp on ACT,
        causal mask via gpsimd affine_select on diagonal chunks,
        O_u/denom accumulate: V_ext.T @ P
    - normalize via reciprocal + PE ones-broadcast, write OT
    - y[s, :] partial = OT.T @ WoT
"""
import numpy as np

S = 4096
D = 1024
NH = 16
DH = 64          # head dim
NCORES = 8
QT_TILES = 8     # 512-wide q/s tiles
KC_PER_TILE = 4  # 128-wide k-chunks per 512 tile

_cached = {}


def _build_nc():
    import concourse.tile as tile
    import concourse.mybir as mybir
    from concourse import bacc

    F32 = mybir.dt.float32
    F32R = mybir.dt.float32r
    EXP = mybir.ActivationFunctionType.Exp

    nc = bacc.Bacc("TRN2", target_bir_lowering=False)
    x_d = nc.dram_tensor("x2d", [S, D], F32, kind="ExternalInput")
    wq_d = nc.dram_tensor("wq", [128, D], F32, kind="ExternalInput")
    wk_d = nc.dram_tensor("wk", [128, D], F32, kind="ExternalInput")
    wv_d = nc.dram_tensor("wv", [128, D], F32, kind="ExternalInput")
    wo_d = nc.dram_tensor("wo", [D, 128], F32, kind="ExternalInput")
    y_d = nc.dram_tensor("y", [S, D], F32, kind="ExternalOutput")

    xr = x_d.bitcast(F32R)

    with tile.TileContext(nc) as tc:
        with (
            tc.tile_pool(name="big", bufs=1) as big,
            tc.tile_pool(name="ps_st", bufs=3, space="PSUM") as ps_st,
            tc.tile_pool(name="ps_tp", bufs=2, space="PSUM") as ps_tp,
            tc.tile_pool(name="ps_ac", bufs=3, space="PSUM") as ps_ac,
            tc.tile_pool(name="xt", bufs=2) as xtp,
            tc.tile_pool(name="xn", bufs=3) as xnp,
            tc.tile_pool(name="pp", bufs=3) as ppool,
            tc.tile_pool(name="misc", bufs=2) as misc,
            tc.tile_pool(name="wload", bufs=1) as wload,
        ):
            # ---- persistent tensors ----
            QT = big.tile([128, S], F32R)      # [2*64 head rows, s]
            KT = big.tile([128, S], F32R)
            VA = big.tile([128, 32, 65], F32R)  # [k%128, kc, d+ones] head A
            VB = big.tile([128, 32, 65], F32R)
            OT = big.tile([128, S], F32R)      # [2*64 d2 rows, s] normalized
            wqt = big.tile([128, 8, 128], F32R)
            wkt = big.tile([128, 8, 128], F32R)
            wvt = big.tile([128, 8, 128], F32R)
            wot = big.tile([128, 2, 512], F32R)  # [d2, n]
            ident = big.tile([128, 128], F32R)
            ones_c = big.tile([128, 64], F32R)

            ident_dram = nc.inline_tensor(np.eye(128, dtype=np.float32), name="ident_c")
            ones_dram = nc.inline_tensor(np.ones((128, 64), np.float32), name="ones_c")
            nc.sync.dma_start(ident[:], ident_dram.bitcast(F32R)[:])
            nc.sync.dma_start(ones_c[:], ones_dram.bitcast(F32R)[:])
            # ones columns of V_ext
            nc.sync.dma_start(VA[:, :, 64:65], ones_dram.bitcast(F32R)[:, 0:32, None])
            nc.sync.dma_start(VB[:, :, 64:65], ones_dram.bitcast(F32R)[:, 0:32, None])

            # ---- load + transpose weights ----
            wq_n = wload.tile([128, 8, 128], F32R, tag="wnat", name="wq_n")
            wk_n = wload.tile([128, 8, 128], F32R, tag="wnat2", name="wk_n")
            wv_n = wload.tile([128, 8, 128], F32R, tag="wnat3", name="wv_n")
            wo_n = wload.tile([128, 8, 128], F32R, tag="wnat4", name="wo_n")
            nc.sync.dma_start(wq_n[:], wq_d.bitcast(F32R).rearrange("m (j d) -> m j d", d=128))
            nc.sync.dma_start(wk_n[:], wk_d.bitcast(F32R).rearrange("m (j d) -> m j d", d=128))
            nc.sync.dma_start(wv_n[:], wv_d.bitcast(F32R).rearrange("m (j d) -> m j d", d=128))
            nc.sync.dma_start(wo_n[:], wo_d.bitcast(F32R).rearrange("(o p) d -> p o d", p=128))
            for src, dst in ((wq_n, wqt), (wk_n, wkt), (wv_n, wvt)):
                for j in range(0, 8, 4):
                    tp = ps_tp.tile([128, 512], F32, tag="tp", name="tp_w")
                    for b in range(4):
                        nc.tensor.transpose(tp[:, 128 * b:128 * (b + 1)].bitcast(F32R),
                                            src[:, j + b, :], ident[:])
                    for b in range(4):
                        nc.vector.tensor_copy(dst[:, j + b, :], tp[:, 128 * b:128 * (b + 1)].bitcast(F32R))
            wotv = wot[:].rearrange("p a (o d) -> p (a o) d", d=128)
            for j in range(0, 8, 4):
                tp = ps_tp.tile([128, 512], F32, tag="tp", name="tp_wo")
                for b in range(4):
                    nc.tensor.transpose(tp[:, 128 * b:128 * (b + 1)].bitcast(F32R),
                                        wo_n[:, j + b, :], ident[:])
                for b in range(4):
                    nc.vector.tensor_copy(wotv[:, j + b, :], tp[:, 128 * b:128 * (b + 1)].bitcast(F32R))

            # ---- main interleaved loop ----
            for i in range(QT_TILES):
                s0 = 512 * i
                # phase 1: xT for s-tile i, then QT/KT/VT columns
                xt_t = xtp.tile([128, 8, 512], F32R, tag="xt", name="xt_t")
                for b in range(4):
                    x_n = xnp.tile([128, 1024], F32R, tag="xn", name="x_n")
                    nc.sync.dma_start(x_n[:], xr[s0 + 128 * b: s0 + 128 * (b + 1), :])
                    for j2 in range(2):
                        tp = ps_tp.tile([128, 512], F32, tag="tp", name="tp_x")
                        for jj in range(4):
                            j = 4 * j2 + jj
                            nc.tensor.transpose(
                                tp[:, 128 * jj:128 * (jj + 1)].bitcast(F32R),
                                x_n[:, 128 * j:128 * (j + 1)], ident[:])
                        for jj in range(4):
                            j = 4 * j2 + jj
                            nc.vector.tensor_copy(
                                xt_t[:, j, 128 * b:128 * (b + 1)],
                                tp[:, 128 * jj:128 * (jj + 1)].bitcast(F32R))
                qp = ps_ac.tile([128, 512], F32, tag="ac", name="qp")
                kp = ps_ac.tile([128, 512], F32, tag="ac", name="kp")
                vp = ps_ac.tile([128, 512], F32, tag="ac", name="vp")
                for j in range(8):
                    nc.tensor.matmul(qp[:], wqt[:, j, :], xt_t[:, j, :],
                                     start=(j == 0), stop=(j == 7))
                    nc.tensor.matmul(kp[:], wkt[:, j, :], xt_t[:, j, :],
                                     start=(j == 0), stop=(j == 7))
                    nc.tensor.matmul(vp[:], wvt[:, j, :], xt_t[:, j, :],
                                     start=(j == 0), stop=(j == 7))
                nc.vector.tensor_copy(QT[:, s0:s0 + 512], qp[:].bitcast(F32R))
                nc.vector.tensor_copy(KT[:, s0:s0 + 512], kp[:].bitcast(F32R))
                vt_t = misc.tile([128, 512], F32R, tag="vt", name="vt_t")
                nc.vector.tensor_copy(vt_t[:], vp[:].bitcast(F32R))
                # transpose VT tile -> V_ext chunks (kc = 4i..4i+3)
                tpv = ps_tp.tile([128, 512], F32, tag="tp", name="tp_v")
                for b in range(4):
                    nc.tensor.transpose(tpv[:, 128 * b:128 * (b + 1)].bitcast(F32R),
                                        vt_t[:, 128 * b:128 * (b + 1)], ident[:])
                for b in range(4):
                    kc = 4 * i + b
                    nc.vector.tensor_copy(VA[:, kc, 0:64],
                                          tpv[:, 128 * b:128 * b + 64].bitcast(F32R))
                    nc.vector.tensor_copy(VB[:, kc, 0:64],
                                          tpv[:, 128 * b + 64:128 * (b + 1)].bitcast(F32R))

                # phase 2: attention for q-tile i
                pvA = ps_st.tile([65, 512], F32, tag="st", name="pvA")
                pvB = ps_st.tile([65, 512], F32, tag="st", name="pvB")
                nkc = KC_PER_TILE * (i + 1)
                for kc in range(nkc):
                    k0 = 128 * kc
                    sA = ps_ac.tile([128, 512], F32, tag="ac", name="sA")
                    sB = ps_ac.tile([128, 512], F32, tag="ac", name="sB")
                    nc.tensor.matmul(sA[:], KT[0:64, k0:k0 + 128], QT[0:64, s0:s0 + 512],
                                     start=True, stop=True)
                    nc.tensor.matmul(sB[:], KT[64:128, k0:k0 + 128], QT[64:128, s0:s0 + 512],
                                     start=True, stop=True)
                    pa = ppool.tile([128, 512], F32R, tag="pa", name="pa")
                    pb = ppool.tile([128, 512], F32R, tag="pb", name="pb")
                    nc.scalar.activation(pa[:], sA[:], EXP, scale=0.125)
                    nc.scalar.activation(pb[:], sB[:], EXP, scale=0.125)
                    doff = kc - 4 * i
                    if doff >= 0:  # diagonal chunk: causal mask
                        for pT in (pa, pb):
                            nc.gpsimd.affine_select(
                                out=pT[:], in_=pT[:],
                                compare_op=mybir.AluOpType.is_ge,
                                fill=0.0, base=-128 * doff,
                                pattern=[[1, 512]], channel_multiplier=-1)
                    nc.tensor.matmul(pvA[0:65], VA[:, kc, :], pa[:],
                                     start=(kc == 0), stop=(kc == nkc - 1))
                    nc.tensor.matmul(pvB[0:65], VB[:, kc, :], pb[:],
                                     start=(kc == 0), stop=(kc == nkc - 1))
                # normalize
                otB = misc.tile([64, 512], F32R, tag="otB", name="otB")
                for h, pvs in ((0, pvA), (1, pvB)):
                    recip = misc.tile([128, 512], F32R, tag="recip", name=f"recip{h}")
                    with nc.allow_low_precision("softmax recip"):
                        nc.vector.reciprocal(recip[64:65], pvs[64:65])
                    bcs = ps_tp.tile([64, 512], F32, tag="tp", name=f"bcs{h}")
                    nc.tensor.matmul(bcs[0:64], ones_c[64:65, :], recip[64:65],
                                     start=True, stop=True)
                    bc_sb = misc.tile([64, 512], F32, tag="bcsb", name=f"bcsb{h}")
                    nc.vector.tensor_copy(bc_sb[:], bcs[:])
                    if h == 0:
                        nc.vector.tensor_mul(OT[0:64, s0:s0 + 512], pvs[0:64], bc_sb[:])
                    else:
                        nc.vector.tensor_mul(otB[:], pvs[0:64], bc_sb[:])
                        nc.sync.dma_start(OT[64:128, s0:s0 + 512], otB[:])

                # phase 3: output projection for s rows of this tile
                for sc in range(4):
                    c0 = s0 + 128 * sc
                    yps = ps_st.tile([128, 512], F32, tag="st", name="yps")
                    yps2 = ps_st.tile([128, 512], F32, tag="st", name="yps2")
                    nc.tensor.matmul(yps[:], OT[:, c0:c0 + 128], wot[:, 0, :],
                                     start=True, stop=True)
                    nc.tensor.matmul(yps2[:], OT[:, c0:c0 + 128], wot[:, 1, :],
                                     start=True, stop=True)
                    y_sb = misc.tile([128, 1024], F32, tag="ysb", name="y_sb")
                    nc.vector.tensor_copy(y_sb[:, 0:512], yps[:])
                    nc.vector.tensor_copy(y_sb[:, 512:1024], yps2[:])
                    nc.sync.dma_start(y_d[c0:c0 + 128, :], y_sb[:])
    nc.finalize()
    return nc


def kernel(x, W_Q, W_K, W_V, W_O):
    import sys
    if '/opt/trn_rl_repo' not in sys.path:
        sys.path.insert(0, '/opt/trn_rl_repo')
    from concourse.bass_utils import run_bass_kernel_spmd

    if "nc" not in _cached:
        _cached["nc"] = _build_nc()
    nc = _cached["nc"]

    x2d = np.ascontiguousarray(np.asarray(x, dtype=np.float32).reshape(S, D))
    W_Q = np.asarray(W_Q, dtype=np.float32)
    W_K = np.asarray(W_K, dtype=np.float32)
    W_V = np.asarray(W_V, dtype=np.float32)
    W_O = np.asarray(W_O, dtype=np.float32)

    in_maps = []
    for c in range(NCORES):
        sl = slice(128 * c, 128 * (c + 1))
        in_maps.append({
            "x2d": x2d,
            "wq": np.ascontiguousarray(W_Q[sl, :]),
            "wk": np.ascontiguousarray(W_K[sl, :]),
            "wv": np.ascontiguousarray(W_V[sl, :]),
            "wo": np.ascontiguousarray(W_O[:, sl]),
        })

    res = run_bass_kernel_spmd(nc, in_maps, core_ids=list(range(NCORES)),
                               trace_cores=[0] if __import__("os").environ.get("BASS_TRACE") else None)
    _cached["last_res"] = res
    y = np.zeros((S, D), dtype=np.float64)
    for c in range(NCORES):
        y += res.results[c]["y"]
    return y.astype(np.float32).reshape(1, S, D)


# revision 3
# speedup vs baseline: 1.1425x; 1.1425x over previous
"""Multi-head self-attention (B=1, S=4096, D=1024, H=16, causal) on 8 TRN2 cores.

Sharding: tensor-parallel over heads. Each core computes 2 heads end-to-end:
QKV projections (column-split), causal attention, and a partial output
projection (W_O row-split). Host sums the 8 partial outputs.

Per-core pipeline (fused, single NEFF, fp32r matmuls), software-prefetched:
phase1(i+1) [x load + PE-transpose + QKV projections] overlaps attention(i)
[row-packed S matmuls, merged 2-head exp on ACT, causal mask on GPSIMD,
V_ext(M=65, ones col -> denominator) PV accumulation] and yproj(i).
"""
import numpy as np

S = 4096
D = 1024
NCORES = 8
QT_TILES = 8     # 512-wide q/s tiles
KC_PER_TILE = 4  # 128-wide k-chunks per 512 tile

_cached = {}


def _build_nc():
    import concourse.tile as tile
    import concourse.mybir as mybir
    from concourse import bacc

    F32 = mybir.dt.float32
    F32R = mybir.dt.float32r
    EXP = mybir.ActivationFunctionType.Exp

    nc = bacc.Bacc("TRN2", target_bir_lowering=False)
    x_d = nc.dram_tensor("x2d", [S, D], F32, kind="ExternalInput")
    wq_d = nc.dram_tensor("wq", [128, D], F32, kind="ExternalInput")
    wk_d = nc.dram_tensor("wk", [128, D], F32, kind="ExternalInput")
    wv_d = nc.dram_tensor("wv", [128, D], F32, kind="ExternalInput")
    wo_d = nc.dram_tensor("wo", [D, 128], F32, kind="ExternalInput")
    y_d = nc.dram_tensor("y", [S, D], F32, kind="ExternalOutput")

    xr = x_d.bitcast(F32R)

    with tile.TileContext(nc) as tc:
        with (
            tc.tile_pool(name="big", bufs=1) as big,
            tc.tile_pool(name="ps_sab", bufs=2, space="PSUM") as ps_sab,
            tc.tile_pool(name="ps_pv", bufs=3, space="PSUM") as ps_pv,
            tc.tile_pool(name="ps_pj", bufs=1, space="PSUM") as ps_pj,
            tc.tile_pool(name="xt", bufs=2) as xtp,
            tc.tile_pool(name="xn", bufs=3) as xnp,
            tc.tile_pool(name="pp", bufs=3) as ppool,
            tc.tile_pool(name="misc", bufs=2) as misc,
            tc.tile_pool(name="wload", bufs=1) as wload,
        ):
            # ---- persistent tensors ----
            QT = big.tile([128, S], F32R)       # [2*64 head rows, s]
            KT = big.tile([128, S], F32R)
            VA = big.tile([128, 32, 65], F32R)  # [k%128, kc, d+ones] head A
            VB = big.tile([128, 32, 65], F32R)
            OT = big.tile([128, S], F32R)       # [2*64 d2 rows, s] normalized
            wqt = big.tile([128, 8, 128], F32R)
            wkt = big.tile([128, 8, 128], F32R)
            wvt = big.tile([128, 8, 128], F32R)
            wot = big.tile([128, 2, 512], F32R)  # [d2, n]
            ident = big.tile([128, 128], F32R)
            ones_c = big.tile([128, 64], F32R)

            ident_dram = nc.inline_tensor(np.eye(128, dtype=np.float32), name="ident_c")
            ones_dram = nc.inline_tensor(np.ones((128, 64), np.float32), name="ones_c")
            nc.sync.dma_start(ident[:], ident_dram.bitcast(F32R)[:])
            nc.sync.dma_start(ones_c[:], ones_dram.bitcast(F32R)[:])
            nc.sync.dma_start(VA[:, :, 64:65], ones_dram.bitcast(F32R)[:, 0:32, None])
            nc.sync.dma_start(VB[:, :, 64:65], ones_dram.bitcast(F32R)[:, 0:32, None])

            # ---- load + transpose weights ----
            wq_n = wload.tile([128, 8, 128], F32R, tag="wnat", name="wq_n")
            wk_n = wload.tile([128, 8, 128], F32R, tag="wnat2", name="wk_n")
            wv_n = wload.tile([128, 8, 128], F32R, tag="wnat3", name="wv_n")
            wo_n = wload.tile([128, 8, 128], F32R, tag="wnat4", name="wo_n")
            nc.sync.dma_start(wq_n[:], wq_d.bitcast(F32R).rearrange("m (j d) -> m j d", d=128))
            nc.sync.dma_start(wk_n[:], wk_d.bitcast(F32R).rearrange("m (j d) -> m j d", d=128))
            nc.sync.dma_start(wv_n[:], wv_d.bitcast(F32R).rearrange("m (j d) -> m j d", d=128))
            nc.sync.dma_start(wo_n[:], wo_d.bitcast(F32R).rearrange("(o p) d -> p o d", p=128))
            wotv = wot[:].rearrange("p a (o d) -> p (a o) d", d=128)
            with nc.named_scope("wtrans"):
                for src, dst in ((wq_n, wqt), (wk_n, wkt), (wv_n, wvt), (wo_n, wotv)):
                    for j in range(0, 8, 4):
                        tp = ps_pj.tile([128, 512], F32, tag="pj", name="tp_w")
                        for b in range(4):
                            nc.tensor.transpose(tp[:, 128 * b:128 * (b + 1)].bitcast(F32R),
                                                src[:, j + b, :], ident[:])
                        for b in range(4):
                            nc.vector.tensor_copy(dst[:, j + b, :],
                                                  tp[:, 128 * b:128 * (b + 1)].bitcast(F32R))

            def phase1(i):
                """x s-tile i -> xT -> QT/KT/VT[:, tile] -> V_ext chunks."""
                s0 = 512 * i
                xt_t = xtp.tile([128, 8, 512], F32R, tag="xt", name="xt_t")
                for b in range(4):
                    x_n = xnp.tile([128, 1024], F32R, tag="xn", name="x_n")
                    nc.sync.dma_start(x_n[:], xr[s0 + 128 * b: s0 + 128 * (b + 1), :])
                    for j2 in range(2):
                        tp = ps_pj.tile([128, 512], F32, tag="pj", name="tp_x")
                        for jj in range(4):
                            j = 4 * j2 + jj
                            nc.tensor.transpose(
                                tp[:, 128 * jj:128 * (jj + 1)].bitcast(F32R),
                                x_n[:, 128 * j:128 * (j + 1)], ident[:])
                        for jj in range(4):
                            j = 4 * j2 + jj
                            nc.vector.tensor_copy(
                                xt_t[:, j, 128 * b:128 * (b + 1)],
                                tp[:, 128 * jj:128 * (jj + 1)].bitcast(F32R))
                for w_t, dst in ((wqt, QT), (wkt, KT)):
                    acc = ps_pj.tile([128, 512], F32, tag="pj", name="acc_qk")
                    for j in range(8):
                        nc.tensor.matmul(acc[:], w_t[:, j, :], xt_t[:, j, :],
                                         start=(j == 0), stop=(j == 7))
                    nc.vector.tensor_copy(dst[:, s0:s0 + 512], acc[:].bitcast(F32R))
                acc = ps_pj.tile([128, 512], F32, tag="pj", name="acc_v")
                for j in range(8):
                    nc.tensor.matmul(acc[:], wvt[:, j, :], xt_t[:, j, :],
                                     start=(j == 0), stop=(j == 7))
                vt_t = misc.tile([128, 512], F32R, tag="vt", name="vt_t")
                nc.vector.tensor_copy(vt_t[:], acc[:].bitcast(F32R))
                tpv = ps_pj.tile([128, 512], F32, tag="pj", name="tp_v")
                for b in range(4):
                    nc.tensor.transpose(tpv[:, 128 * b:128 * (b + 1)].bitcast(F32R),
                                        vt_t[:, 128 * b:128 * (b + 1)], ident[:])
                for b in range(4):
                    kc = 4 * i + b
                    nc.vector.tensor_copy(VA[:, kc, 0:64],
                                          tpv[:, 128 * b:128 * b + 64].bitcast(F32R))
                    nc.vector.tensor_copy(VB[:, kc, 0:64],
                                          tpv[:, 128 * b + 64:128 * (b + 1)].bitcast(F32R))

            def attention(i):
                s0 = 512 * i
                pvA = ps_pv.tile([65, 512], F32, tag="pv", name="pvA")
                pvB = ps_pv.tile([65, 512], F32, tag="pv", name="pvB")
                nkc = KC_PER_TILE * (i + 1)
                for kc in range(nkc):
                    k0 = 128 * kc
                    sab = ps_sab.tile([128, 1024], F32, tag="sab", name="sab")
                    nc.tensor.matmul(sab[:, 0:512], KT[0:64, k0:k0 + 128],
                                     QT[0:64, s0:s0 + 512], start=True, stop=True)
                    nc.tensor.matmul(sab[:, 512:1024], KT[64:128, k0:k0 + 128],
                                     QT[64:128, s0:s0 + 512], start=True, stop=True)
                    p2 = ppool.tile([128, 1024], F32R, tag="p2", name="p2")
                    nc.scalar.activation(p2[:], sab[:], EXP, scale=0.125)
                    doff = kc - 4 * i
                    if doff >= 0:  # diagonal chunk: causal mask both halves
                        nc.gpsimd.affine_select(
                            out=p2[:].rearrange("p (h q) -> p h q", h=2),
                            in_=p2[:].rearrange("p (h q) -> p h q", h=2),
                            compare_op=mybir.AluOpType.is_ge,
                            fill=0.0, base=-128 * doff,
                            pattern=[[0, 2], [1, 512]], channel_multiplier=-1)
                    nc.tensor.matmul(pvA[0:65], VA[:, kc, :], p2[:, 0:512],
                                     start=(kc == 0), stop=(kc == nkc - 1))
                    nc.tensor.matmul(pvB[0:65], VB[:, kc, :], p2[:, 512:1024],
                                     start=(kc == 0), stop=(kc == nkc - 1))
                # normalize
                otB = misc.tile([64, 512], F32R, tag="otB", name="otB")
                for h, pvs in ((0, pvA), (1, pvB)):
                    recip = misc.tile([128, 512], F32R, tag="recip", name=f"recip{h}")
                    with nc.allow_low_precision("softmax recip"):
                        nc.vector.reciprocal(recip[64:65], pvs[64:65])
                    bcs = ps_pj.tile([64, 512], F32, tag="pj", name=f"bcs{h}")
                    nc.tensor.matmul(bcs[0:64], ones_c[64:65, :], recip[64:65],
                                     start=True, stop=True)
                    bc_sb = misc.tile([64, 512], F32, tag="bcsb", name=f"bcsb{h}")
                    nc.vector.tensor_copy(bc_sb[:], bcs[:])
                    if h == 0:
                        nc.vector.tensor_mul(OT[0:64, s0:s0 + 512], pvs[0:64], bc_sb[:])
                    else:
                        nc.vector.tensor_mul(otB[:], pvs[0:64], bc_sb[:])
                        nc.sync.dma_start(OT[64:128, s0:s0 + 512], otB[:])

            def yproj(i):
                s0 = 512 * i
                for sc in range(4):
                    c0 = s0 + 128 * sc
                    yps = ps_pv.tile([128, 512], F32, tag="pv", name="yps")
                    yps2 = ps_pv.tile([128, 512], F32, tag="pv", name="yps2")
                    nc.tensor.matmul(yps[:], OT[:, c0:c0 + 128], wot[:, 0, :],
                                     start=True, stop=True)
                    nc.tensor.matmul(yps2[:], OT[:, c0:c0 + 128], wot[:, 1, :],
                                     start=True, stop=True)
                    y_sb = misc.tile([128, 1024], F32, tag="ysb", name="y_sb")
                    nc.vector.tensor_copy(y_sb[:, 0:512], yps[:])
                    nc.vector.tensor_copy(y_sb[:, 512:1024], yps2[:])
                    nc.sync.dma_start(y_d[c0:c0 + 128, :], y_sb[:])

            # ---- main loop, software-prefetched ----
            with nc.named_scope("p1_0"):
                phase1(0)
            for i in range(QT_TILES):
                if i + 1 < QT_TILES:
                    with nc.named_scope(f"p1_{i + 1}"):
                        phase1(i + 1)
                with nc.named_scope(f"att_{i}"):
                    attention(i)
                with nc.named_scope(f"yp_{i}"):
                    yproj(i)
    nc.finalize()
    return nc


def kernel(x, W_Q, W_K, W_V, W_O):
    import sys
    if '/opt/trn_rl_repo' not in sys.path:
        sys.path.insert(0, '/opt/trn_rl_repo')
    from concourse.bass_utils import run_bass_kernel_spmd

    if "nc" not in _cached:
        _cached["nc"] = _build_nc()
    nc = _cached["nc"]

    x2d = np.ascontiguousarray(np.asarray(x, dtype=np.float32).reshape(S, D))
    W_Q = np.asarray(W_Q, dtype=np.float32)
    W_K = np.asarray(W_K, dtype=np.float32)
    W_V = np.asarray(W_V, dtype=np.float32)
    W_O = np.asarray(W_O, dtype=np.float32)

    in_maps = []
    for c in range(NCORES):
        sl = slice(128 * c, 128 * (c + 1))
        in_maps.append({
            "x2d": x2d,
            "wq": np.ascontiguousarray(W_Q[sl, :]),
            "wk": np.ascontiguousarray(W_K[sl, :]),
            "wv": np.ascontiguousarray(W_V[sl, :]),
            "wo": np.ascontiguousarray(W_O[:, sl]),
        })

    import os
    res = run_bass_kernel_spmd(nc, in_maps, core_ids=list(range(NCORES)),
                               trace_cores=[0] if os.environ.get("BASS_TRACE") else None)
    _cached["last_res"] = res
    y = np.zeros((S, D), dtype=np.float64)
    for c in range(NCORES):
        y += res.results[c]["y"]
    return y.astype(np.float32).reshape(1, S, D)


# revision 4
# speedup vs baseline: 1.1575x; 1.0131x over previous
"""Multi-head self-attention (B=1, S=4096, D=1024, H=16, causal) on 8 TRN2 cores.

Sharding: tensor-parallel over heads. Each core computes 2 heads end-to-end:
QKV projections (column-split), causal attention, and a partial output
projection (W_O row-split). Host sums the 8 partial outputs.

Per-core pipeline (fused, single NEFF, fp32r matmuls), software-prefetched:
phase1(i+1) [x load + PE-transpose + QKV projections] overlaps attention(i)
[row-packed S matmuls, merged 2-head exp on ACT, causal mask on GPSIMD,
V_ext(M=65, ones col -> denominator) PV accumulation] and yproj(i).
"""
import numpy as np

S = 4096
D = 1024
NCORES = 8
QT_TILES = 8     # 512-wide q/s tiles
KC_PER_TILE = 4  # 128-wide k-chunks per 512 tile

_cached = {}


def _build_nc():
    import concourse.tile as tile
    import concourse.mybir as mybir
    from concourse import bacc

    F32 = mybir.dt.float32
    F32R = mybir.dt.float32r
    EXP = mybir.ActivationFunctionType.Exp

    nc = bacc.Bacc("TRN2", target_bir_lowering=False)
    x_d = nc.dram_tensor("x2d", [S, D], F32, kind="ExternalInput")
    wq_d = nc.dram_tensor("wq", [128, D], F32, kind="ExternalInput")
    wk_d = nc.dram_tensor("wk", [128, D], F32, kind="ExternalInput")
    wv_d = nc.dram_tensor("wv", [128, D], F32, kind="ExternalInput")
    wo_d = nc.dram_tensor("wo", [D, 128], F32, kind="ExternalInput")
    y_d = nc.dram_tensor("y", [S, D], F32, kind="ExternalOutput")

    xr = x_d.bitcast(F32R)

    with tile.TileContext(nc) as tc:
        with (
            tc.tile_pool(name="big", bufs=1) as big,
            tc.tile_pool(name="ps_sab", bufs=2, space="PSUM") as ps_sab,  # sa+sb tags
            tc.tile_pool(name="ps_pv", bufs=2, space="PSUM") as ps_pv,
            tc.tile_pool(name="ps_pj", bufs=2, space="PSUM") as ps_pj,
            tc.tile_pool(name="xt", bufs=2) as xtp,
            tc.tile_pool(name="xn", bufs=3) as xnp,
            tc.tile_pool(name="pp", bufs=3) as ppool,
            tc.tile_pool(name="misc", bufs=2) as misc,
            tc.tile_pool(name="wload", bufs=1) as wload,
        ):
            # ---- persistent tensors ----
            QT = big.tile([128, S], F32R)       # [2*64 head rows, s]
            KT = big.tile([128, S], F32R)
            VA = big.tile([128, 32, 65], F32R)  # [k%128, kc, d+ones] head A
            VB = big.tile([128, 32, 65], F32R)
            OT = big.tile([128, S], F32R)       # [2*64 d2 rows, s] normalized
            wqt = big.tile([128, 8, 128], F32R)
            wkt = big.tile([128, 8, 128], F32R)
            wvt = big.tile([128, 8, 128], F32R)
            wot = big.tile([128, 2, 512], F32R)  # [d2, n]
            ident = big.tile([128, 128], F32R)
            ones_c = big.tile([128, 64], F32R)

            ident_dram = nc.inline_tensor(np.eye(128, dtype=np.float32), name="ident_c")
            ones_dram = nc.inline_tensor(np.ones((128, 64), np.float32), name="ones_c")
            nc.sync.dma_start(ident[:], ident_dram.bitcast(F32R)[:])
            nc.sync.dma_start(ones_c[:], ones_dram.bitcast(F32R)[:])
            nc.sync.dma_start(VA[:, :, 64:65], ones_dram.bitcast(F32R)[:, 0:32, None])
            nc.sync.dma_start(VB[:, :, 64:65], ones_dram.bitcast(F32R)[:, 0:32, None])

            # ---- load + transpose weights ----
            wq_n = wload.tile([128, 8, 128], F32R, tag="wnat", name="wq_n")
            wk_n = wload.tile([128, 8, 128], F32R, tag="wnat2", name="wk_n")
            wv_n = wload.tile([128, 8, 128], F32R, tag="wnat3", name="wv_n")
            wo_n = wload.tile([128, 8, 128], F32R, tag="wnat4", name="wo_n")
            nc.sync.dma_start(wq_n[:], wq_d.bitcast(F32R).rearrange("m (j d) -> m j d", d=128))
            nc.sync.dma_start(wk_n[:], wk_d.bitcast(F32R).rearrange("m (j d) -> m j d", d=128))
            nc.sync.dma_start(wv_n[:], wv_d.bitcast(F32R).rearrange("m (j d) -> m j d", d=128))
            nc.sync.dma_start(wo_n[:], wo_d.bitcast(F32R).rearrange("(o p) d -> p o d", p=128))
            wotv = wot[:].rearrange("p a (o d) -> p (a o) d", d=128)
            with nc.named_scope("wtrans"):
                for src, dst in ((wq_n, wqt), (wk_n, wkt), (wv_n, wvt), (wo_n, wotv)):
                    for j in range(0, 8, 4):
                        tp = ps_pj.tile([128, 512], F32, tag="pj", name="tp_w")
                        for b in range(4):
                            nc.tensor.transpose(tp[:, 128 * b:128 * (b + 1)].bitcast(F32R),
                                                src[:, j + b, :], ident[:])
                        for b in range(4):
                            nc.vector.tensor_copy(dst[:, j + b, :],
                                                  tp[:, 128 * b:128 * (b + 1)].bitcast(F32R))

            def phase1(i):
                """x s-tile i -> xT -> QT/KT/VT[:, tile] -> V_ext chunks."""
                s0 = 512 * i
                xt_t = xtp.tile([128, 8, 512], F32R, tag="xt", name="xt_t")
                for b in range(4):
                    x_n = xnp.tile([128, 1024], F32R, tag="xn", name="x_n")
                    nc.sync.dma_start(x_n[:], xr[s0 + 128 * b: s0 + 128 * (b + 1), :])
                    for j2 in range(2):
                        tp = ps_pj.tile([128, 512], F32, tag="pj", name="tp_x")
                        for jj in range(4):
                            j = 4 * j2 + jj
                            nc.tensor.transpose(
                                tp[:, 128 * jj:128 * (jj + 1)].bitcast(F32R),
                                x_n[:, 128 * j:128 * (j + 1)], ident[:])
                        for jj in range(4):
                            j = 4 * j2 + jj
                            nc.vector.tensor_copy(
                                xt_t[:, j, 128 * b:128 * (b + 1)],
                                tp[:, 128 * jj:128 * (jj + 1)].bitcast(F32R))
                for w_t, dst in ((wqt, QT), (wkt, KT)):
                    acc = ps_pj.tile([128, 512], F32, tag="pj", name="acc_qk")
                    for j in range(8):
                        nc.tensor.matmul(acc[:], w_t[:, j, :], xt_t[:, j, :],
                                         start=(j == 0), stop=(j == 7))
                    nc.vector.tensor_copy(dst[:, s0:s0 + 512], acc[:].bitcast(F32R))
                acc = ps_pj.tile([128, 512], F32, tag="pj", name="acc_v")
                for j in range(8):
                    nc.tensor.matmul(acc[:], wvt[:, j, :], xt_t[:, j, :],
                                     start=(j == 0), stop=(j == 7))
                vt_t = misc.tile([128, 512], F32R, tag="vt", name="vt_t")
                nc.vector.tensor_copy(vt_t[:], acc[:].bitcast(F32R))
                tpv = ps_pj.tile([128, 512], F32, tag="pj", name="tp_v")
                for b in range(4):
                    nc.tensor.transpose(tpv[:, 128 * b:128 * (b + 1)].bitcast(F32R),
                                        vt_t[:, 128 * b:128 * (b + 1)], ident[:])
                for b in range(4):
                    kc = 4 * i + b
                    nc.vector.tensor_copy(VA[:, kc, 0:64],
                                          tpv[:, 128 * b:128 * b + 64].bitcast(F32R))
                    nc.vector.tensor_copy(VB[:, kc, 0:64],
                                          tpv[:, 128 * b + 64:128 * (b + 1)].bitcast(F32R))

            def attention(i):
                s0 = 512 * i
                pvA = ps_pv.tile([65, 512], F32, tag="pv", name="pvA")
                pvB = ps_pv.tile([65, 512], F32, tag="pv", name="pvB")
                nkc = KC_PER_TILE * (i + 1)
                for kc in range(nkc):
                    k0 = 128 * kc
                    sA = ps_sab.tile([128, 512], F32, tag="sa", name="sA")
                    sB = ps_sab.tile([128, 512], F32, tag="sb", name="sB")
                    nc.tensor.matmul(sA[:], KT[0:64, k0:k0 + 128],
                                     QT[0:64, s0:s0 + 512], start=True, stop=True)
                    nc.tensor.matmul(sB[:], KT[64:128, k0:k0 + 128],
                                     QT[64:128, s0:s0 + 512], start=True, stop=True)
                    pa = ppool.tile([128, 512], F32R, tag="pa", name="pa")
                    pb = ppool.tile([128, 512], F32R, tag="pb", name="pb")
                    nc.scalar.activation(pa[:], sA[:], EXP, scale=0.125)
                    nc.scalar.activation(pb[:], sB[:], EXP, scale=0.125)
                    doff = kc - 4 * i
                    if doff >= 0:  # diagonal chunk: causal mask
                        for pT in (pa, pb):
                            nc.gpsimd.affine_select(
                                out=pT[:], in_=pT[:],
                                compare_op=mybir.AluOpType.is_ge,
                                fill=0.0, base=-128 * doff,
                                pattern=[[1, 512]], channel_multiplier=-1)
                    nc.tensor.matmul(pvA[0:65], VA[:, kc, :], pa[:],
                                     start=(kc == 0), stop=(kc == nkc - 1))
                    nc.tensor.matmul(pvB[0:65], VB[:, kc, :], pb[:],
                                     start=(kc == 0), stop=(kc == nkc - 1))
                # normalize
                otB = misc.tile([64, 512], F32R, tag="otB", name="otB")
                for h, pvs in ((0, pvA), (1, pvB)):
                    recip = misc.tile([128, 512], F32R, tag="recip", name=f"recip{h}")
                    with nc.allow_low_precision("softmax recip"):
                        nc.vector.reciprocal(recip[64:65], pvs[64:65])
                    bcs = ps_pj.tile([64, 512], F32, tag="pj", name=f"bcs{h}")
                    nc.tensor.matmul(bcs[0:64], ones_c[64:65, :], recip[64:65],
                                     start=True, stop=True)
                    bc_sb = misc.tile([64, 512], F32, tag="bcsb", name=f"bcsb{h}")
                    nc.vector.tensor_copy(bc_sb[:], bcs[:])
                    if h == 0:
                        nc.vector.tensor_mul(OT[0:64, s0:s0 + 512], pvs[0:64], bc_sb[:])
                    else:
                        nc.vector.tensor_mul(otB[:], pvs[0:64], bc_sb[:])
                        nc.sync.dma_start(OT[64:128, s0:s0 + 512], otB[:])

            def yproj(i):
                s0 = 512 * i
                for sc in range(4):
                    c0 = s0 + 128 * sc
                    yps = ps_pv.tile([128, 512], F32, tag="pv", name="yps")
                    yps2 = ps_pv.tile([128, 512], F32, tag="pv", name="yps2")
                    nc.tensor.matmul(yps[:], OT[:, c0:c0 + 128], wot[:, 0, :],
                                     start=True, stop=True)
                    nc.tensor.matmul(yps2[:], OT[:, c0:c0 + 128], wot[:, 1, :],
                                     start=True, stop=True)
                    y_sb = misc.tile([128, 1024], F32, tag="ysb", name="y_sb")
                    nc.vector.tensor_copy(y_sb[:, 0:512], yps[:])
                    nc.vector.tensor_copy(y_sb[:, 512:1024], yps2[:])
                    nc.sync.dma_start(y_d[c0:c0 + 128, :], y_sb[:])

            # ---- main loop, software-prefetched ----
            with nc.named_scope("p1_0"):
                phase1(0)
            for i in range(QT_TILES):
                if i + 1 < QT_TILES:
                    with nc.named_scope(f"p1_{i + 1}"):
                        phase1(i + 1)
                with nc.named_scope(f"att_{i}"):
                    attention(i)
                with nc.named_scope(f"yp_{i}"):
                    yproj(i)
    nc.finalize()
    return nc


def kernel(x, W_Q, W_K, W_V, W_O):
    import sys
    if '/opt/trn_rl_repo' not in sys.path:
        sys.path.insert(0, '/opt/trn_rl_repo')
    from concourse.bass_utils import run_bass_kernel_spmd

    if "nc" not in _cached:
        _cached["nc"] = _build_nc()
    nc = _cached["nc"]

    x2d = np.ascontiguousarray(np.asarray(x, dtype=np.float32).reshape(S, D))
    W_Q = np.asarray(W_Q, dtype=np.float32)
    W_K = np.asarray(W_K, dtype=np.float32)
    W_V = np.asarray(W_V, dtype=np.float32)
    W_O = np.asarray(W_O, dtype=np.float32)

    in_maps = []
    for c in range(NCORES):
        sl = slice(128 * c, 128 * (c + 1))
        in_maps.append({
            "x2d": x2d,
            "wq": np.ascontiguousarray(W_Q[sl, :]),
            "wk": np.ascontiguousarray(W_K[sl, :]),
            "wv": np.ascontiguousarray(W_V[sl, :]),
            "wo": np.ascontiguousarray(W_O[:, sl]),
        })

    import os
    res = run_bass_kernel_spmd(nc, in_maps, core_ids=list(range(NCORES)),
                               trace_cores=[0] if os.environ.get("BASS_TRACE") else None)
    _cached["last_res"] = res
    y = np.zeros((S, D), dtype=np.float64)
    for c in range(NCORES):
        y += res.results[c]["y"]
    return y.astype(np.float32).reshape(1, S, D)
